# revision 4
# baseline (speedup 1.0000x reference)
"""Trainium2 Bass kernel v2 for nn_Block_19069654794616 (dense transformer).

B=2, S=2048, D=1600, 25 heads x 64, causal, 4x MLP (tanh-gelu), pre-LN.

Distribution (8 cores, token-parallel, zero collectives):
  Core j owns 512 query tokens: chunk A = seq0 super-chunk j (256 tok),
  chunk B = seq1 super-chunk 7-j (mirrored pairing -> every core's
  causal prefix is exactly 9 super-chunks = 2304 tokens).
  Instead of gathering k/v, each core RECOMPUTES k/v for the 9
  super-chunks it attends over, streaming one 256-token super-chunk per
  iteration (LN -> k,v projection -> QK -> exp -> AV) from a
  host-supplied x_ext. Iteration order: [A-diag] + [A full supers] +
  [B full supers] + [B-diag], so the causal triangle mask applies only
  at the two static diagonal iterations. The A/B accumulator switch is
  handled with a host 0/1 reset column (acc = acc*m + av), a predicated
  DMA captures chunk A's result at the per-core switch point, and the
  query-chunk selection per iteration is a dynamic-offset DMA from a
  per-core host offset table. All GEMMs run fp16 (fp32 accumulate).
"""

import numpy as np

import concourse.bass as bass
import concourse.mybir as mybir
import concourse.tile as tile
from concourse import bacc
from concourse.bass_utils import run_bass_kernel_spmd
from concourse.masks import make_identity

f32 = mybir.dt.float32
f16 = mybir.dt.float16
i32 = mybir.dt.int32

N_CORES = 8
B, S, D = 2, 2048, 1600
H, C = 25, 64
D4 = 4 * D
TOK = 512          # query tokens per core
CH = 256           # tokens per super-chunk
NI = 9             # attention iterations (super-chunks in the prefix)
EPS = 1e-5
NHP = 13           # 128-wide head-pair tiles (12 pairs + head 24)
NV = 5             # v column tiles of 325 (= 5 heads x 65)
VW = 325
V65 = 25 * 65      # padded v width (ones column per head)
NJ = [(j * 400, 400) for j in range(4)]
DCH = [(t * 128, 128) for t in range(12)] + [(1536, 64)]
QROW = NHP * CH    # 3328, per-partition row of qdram/qstage
GRP = [17, 17, 16]


def _build():
    nc = bacc.Bacc(
        "TRN2",
        target_bir_lowering=False,
        debug=False,
        enable_asserts=True,
        num_devices=N_CORES,
    )
    t = {}
    t["xq"] = nc.dram_tensor("xq", [TOK, D], f32, kind="ExternalInput").ap()
    t["xext"] = nc.dram_tensor("xext", [NI * CH, D], f16,
                               kind="ExternalInput").ap()
    t["wq"] = nc.dram_tensor("wq", [NHP, 128, NHP, 128], f16,
                             kind="ExternalInput").ap()
    t["wk"] = nc.dram_tensor("wk", [NHP, 128, NHP, 128], f16,
                             kind="ExternalInput").ap()
    t["wv"] = nc.dram_tensor("wv", [NV, 128, NHP, VW], f16,
                             kind="ExternalInput").ap()
    t["wp"] = nc.dram_tensor("wp", [4, 128, NHP, 400], f16,
                             kind="ExternalInput").ap()
    t["wfc"] = nc.dram_tensor("wfc", [50, 128, NHP, 128], f16,
                              kind="ExternalInput").ap()
    t["wo"] = nc.dram_tensor("wo", [4, 50, 128, 400], f16,
                             kind="ExternalInput").ap()
    t["bq"] = nc.dram_tensor("bq", [128, NHP], f32, kind="ExternalInput").ap()
    t["bk"] = nc.dram_tensor("bk", [128, NHP], f32, kind="ExternalInput").ap()
    t["bv"] = nc.dram_tensor("bv", [1, V65], f16, kind="ExternalInput").ap()
    t["bpj"] = nc.dram_tensor("bpj", [1, D], f32, kind="ExternalInput").ap()
    t["bfc"] = nc.dram_tensor("bfc", [128, 50], f32,
                              kind="ExternalInput").ap()
    t["bo"] = nc.dram_tensor("bo", [1, D], f32, kind="ExternalInput").ap()
    t["tri"] = nc.dram_tensor("tri", [128, 2, CH], f16,
                              kind="ExternalInput").ap()
    t["mtab"] = nc.dram_tensor("mtab", [65, NI], f32,
                               kind="ExternalInput").ap()
    t["ctab"] = nc.dram_tensor("ctab", [1, NI], i32,
                               kind="ExternalInput").ap()
    t["qoff"] = nc.dram_tensor("qoff", [1, NI], i32,
                               kind="ExternalInput").ap()
    t["out"] = nc.dram_tensor("out", [TOK, D], f32, kind="ExternalOutput").ap()
    t["qdram"] = nc.dram_tensor("qdram", [2, 128, QROW], f16,
                                kind="Internal").ap()
    t["acap"] = nc.dram_tensor("acap", [65, 25 * CH], f32,
                               kind="Internal").ap()

    with tile.TileContext(nc, pool_alloc_mode="queue") as tc:
        _emit(tc, nc, t)
    nc.compile()
    return nc


def _emit(tc, nc, t):
    sync, vec, act, gp, te = nc.sync, nc.vector, nc.scalar, nc.gpsimd, nc.tensor
    AluOp = mybir.AluOpType
    Act = mybir.ActivationFunctionType

    # ---------------- persistent tiles ----------------
    persist = tc.alloc_tile_pool(name="persist", bufs=1)
    ident = persist.tile([128, 128], f32, name="ident")
    make_identity(nc, ident)
    ident_h = persist.tile([128, 128], f16, name="ident_h")
    vec.tensor_copy(out=ident_h, in_=ident)
    eps_t = persist.tile([128, 1], f32, name="eps_t")
    vec.memset(eps_t, EPS)
    ones_h = persist.tile([1, 64], f16, name="ones_h")
    vec.memset(ones_h, 1.0)
    bq_sb = persist.tile([128, NHP], f32, name="bq_sb")
    sync.dma_start(out=bq_sb, in_=t["bq"])
    bk_sb = persist.tile([128, NHP], f32, name="bk_sb")
    sync.dma_start(out=bk_sb, in_=t["bk"])
    bfc_sb = persist.tile([128, 50], f32, name="bfc_sb")
    sync.dma_start(out=bfc_sb, in_=t["bfc"])
    bv_sb = persist.tile([128, V65], f16, name="bv_sb")
    sync.dma_start(out=bv_sb, in_=bass.AP(
        tensor=t["bv"].tensor, offset=0, ap=[[0, 128], [1, V65]]))
    tri_sb = persist.tile([128, 2, CH], f16, name="tri_sb")
    sync.dma_start(out=tri_sb, in_=t["tri"])
    mtab_sb = persist.tile([65, NI], f32, name="mtab_sb")
    sync.dma_start(out=mtab_sb, in_=t["mtab"])
    ctab_sb = persist.tile([1, NI], i32, name="ctab_sb")
    sync.dma_start(out=ctab_sb, in_=t["ctab"])
    qoff_sb = persist.tile([1, NI], i32, name="qoff_sb")
    sync.dma_start(out=qoff_sb, in_=t["qoff"])

    # one-bank ring of 8 transpose slots (per-tile PSUM alloc is
    # bank-rounded, so small tiles are packed manually)
    psT = tc.alloc_tile_pool(name="psT", bufs=1, space="PSUM")
    tp_ring = psT.tile([128, 8, 128], f16, name="tp_ring")
    tp_ctr = [0]

    def layernorm_cast(pool, xt, label, out=None):
        """LN of a (128, D) tile -> fp16 (128, D) tile (in-place if out
        is given)."""
        stats = pool.tile([128, 4, 6], f32, name=f"{label}st", tag=f"{label}st")
        xg = xt.rearrange("p (g d) -> p g d", g=4)
        for g in range(4):
            vec.bn_stats(out=stats[:, g, :], in_=xg[:, g, :])
        mv = pool.tile([128, 2], f32, name=f"{label}mv", tag=f"{label}mv")
        vec.bn_aggr(out=mv, in_=stats)
        rstd = pool.tile([128, 1], f32, name=f"{label}rs", tag=f"{label}rs")
        act.activation(out=rstd, in_=mv[:, 1:2], func=Act.Sqrt, bias=eps_t)
        vec.reciprocal(out=rstd, in_=rstd)
        xc = out
        if xc is None:
            xc = pool.tile([128, D], f16, name=f"{label}xc",
                           tag=f"{label}xc")
        vec.tensor_scalar(out=xc, in0=xt, scalar1=mv[:, 0:1], scalar2=rstd,
                          op0=AluOp.subtract, op1=AluOp.mult)
        return xc

    def transpose_into(xc, dst, col0):
        """Transpose fp16 (128, D) into dst tiles/slices at column col0."""
        for ci, (d0, dp) in enumerate(DCH):
            s = tp_ctr[0] % 8
            tp_ctr[0] += 1
            tp = tp_ring[:, s, :]
            te.transpose(tp[:dp, :], xc[:, d0:d0 + dp], ident_h)
            vec.tensor_copy(out=dst(ci)[:dp, col0:col0 + 128], in_=tp[:dp, :])

    # long-lived cross-phase pools go on the right-side stack so each
    # side's pool lifetimes nest properly (releases are LIFO per side)
    pool_acc = tc.alloc_tile_pool(name="pool_acc", bufs=1, side="right")
    acc = pool_acc.tile([65, 25, CH], f32, name="acc")
    capA = pool_acc.tile([65, 25, CH], f32, name="capA")
    vec.memset(acc.rearrange("p h q -> p (h q)"), 0.0)
    pool_wkv = tc.alloc_tile_pool(name="pool_wkv", bufs=1)
    wk_sb = [pool_wkv.tile([128, NHP, 128], f16, name=f"wk{i}")
             for i in range(NHP)]
    wv_sb = [pool_wkv.tile([128, NHP, VW], f16, name=f"wv{i}")
             for i in range(NV)]
    # ================= P1: q path =================
    pool_s1 = tc.alloc_tile_pool(name="pool_s1", bufs=1)
    xqT = [pool_s1.tile([128, TOK], f16, name=f"xqT{i}", tag=f"xqT{i}")
           for i in range(NHP)]
    pool_ln = tc.alloc_tile_pool(name="pool_ln", bufs=2)
    psQ = tc.alloc_tile_pool(name="psQ", bufs=2, space="PSUM")

    for tt in range(4):
        xt = pool_ln.tile([128, D], f32, name="lnx", tag="lnx")
        sync.dma_start(out=xt, in_=t["xq"][tt * 128:(tt + 1) * 128, :])
        xc = layernorm_cast(pool_ln, xt, "ln1")
        transpose_into(xc, lambda ci: xqT[ci], tt * 128)
    # k/v weights preload now (after the x loads, so LN starts immediately)
    for i in range(NHP):
        sync.dma_start(out=wk_sb[i], in_=t["wk"][i])
    for i in range(NV):
        sync.dma_start(out=wv_sb[i], in_=t["wv"][i])

    pool_wq = tc.alloc_tile_pool(name="pool_wq", bufs=3)
    pool_qt = tc.alloc_tile_pool(name="pool_qt", bufs=2)
    for tq in range(NHP):
        wt = pool_wq.tile([128, NHP, 128], f16, name=f"wq{tq}", tag="wqk")
        sync.dma_start(out=wt, in_=t["wq"][tq])
        ps = psQ.tile([128, TOK], f32, name="psq", tag="psq")
        for ci, (d0, dp) in enumerate(DCH):
            te.matmul(ps, lhsT=wt[:dp, ci, :], rhs=xqT[ci][:dp, :],
                      start=(ci == 0), stop=(ci == len(DCH) - 1))
        qt = pool_qt.tile([128, TOK], f16, name="qt", tag="qt")
        vec.tensor_scalar(out=qt, in0=ps, scalar1=bq_sb[:, tq:tq + 1],
                          scalar2=None, op0=AluOp.add)
        for c in range(2):
            sync.dma_start(
                out=bass.AP(tensor=t["qdram"].tensor,
                            offset=c * 128 * QROW + tq * CH,
                            ap=[[QROW, 128], [1, CH]]),
                in_=qt[:, c * CH:(c + 1) * CH])
    pool_qt.release()
    pool_wq.release()
    pool_ln.release()
    pool_s1.release()
    psQ.release()

    # ================= P2: attention stream =================
    pool_it = tc.alloc_tile_pool(name="pool_it", bufs=2)
    pool_sc = tc.alloc_tile_pool(name="pool_sc", bufs=2)
    psS = tc.alloc_tile_pool(name="psS", bufs=2, space="PSUM")
    psA = tc.alloc_tile_pool(name="psA", bufs=1, space="PSUM")
    av_ring = psA.tile([65, 2, CH], f32, name="av_ring")   # 1 bank, 2 slots
    psK = tc.alloc_tile_pool(name="psK", bufs=1, space="PSUM")
    psk_ring = psK.tile([128, 2, CH], f32, name="psk_ring")  # 1 bank, 2 slots
    psv_t = psK.tile([128, VW], f32, name="psv_t")           # 1 bank

    def prep(i):
        """Load + LN + transpose super-chunk i's x into a fresh xceT."""
        xceT = pool_it.tile([128, NHP, CH], f16, name="xceT", tag="xceT")
        for t2 in range(2):
            r0 = (i * 2 + t2) * 128
            xe = pool_it.tile([128, D], f16, name="xe", tag="xe")
            act.dma_start(out=xe, in_=t["xext"][r0:r0 + 128, :])
            layernorm_cast(pool_it, xe, "lne", out=xe)
            transpose_into(xe, lambda ci: xceT[:, ci, :], t2 * 128)
        return xceT

    xceT_next = prep(0)
    for i in range(NI):
        xceT = xceT_next
        # dynamic q-chunk stage (offset 0 or 128*QROW per host table)
        qreg = sync.alloc_register(f"qo{i}")
        sync.reg_load(qreg, qoff_sb[0:1, i:i + 1])
        qov = sync.snap(qreg, donate=True, min_val=0, max_val=128 * QROW)
        qst = pool_it.tile([128, NHP, CH], f16, name="qst", tag="qst")
        sync.dma_start(out=qst, in_=bass.AP(
            tensor=t["qdram"].tensor, offset=qov,
            ap=[[QROW, 128], [CH, NHP], [1, CH]]))

        # k projection for this super-chunk
        kt = pool_it.tile([128, NHP, CH], f16, name="kt", tag="kt")
        for tk in range(NHP):
            ps = psk_ring[:, tk % 2, :]
            for ci, (d0, dp) in enumerate(DCH):
                te.matmul(ps, lhsT=wk_sb[tk][:dp, ci, :],
                          rhs=xceT[:dp, ci, :],
                          start=(ci == 0), stop=(ci == len(DCH) - 1))
            vec.tensor_scalar(out=kt[:, tk, :], in0=ps,
                              scalar1=bk_sb[:, tk:tk + 1], scalar2=None,
                              op0=AluOp.add)

        # v projection (padded 65-wide heads; col 64 = 1 via bias)
        vt = pool_it.tile([128, 2, 25, 65], f16, name="vt", tag="vt")
        for t2 in range(2):
            for jv in range(NV):
                ps = psv_t
                for ci, (d0, dp) in enumerate(DCH):
                    te.matmul(ps, lhsT=xceT[:dp, ci, t2 * 128:(t2 + 1) * 128],
                              rhs=wv_sb[jv][:dp, ci, :],
                              start=(ci == 0), stop=(ci == len(DCH) - 1))
                vec.tensor_tensor(
                    out=vt[:, t2, jv * 5:(jv + 1) * 5, :],
                    in0=ps.rearrange("p (h c) -> p h c", h=5),
                    in1=bv_sb.rearrange("p (h c) -> p h c", h=25)[
                        :, jv * 5:(jv + 1) * 5, :],
                    op=AluOp.add)

        if i + 1 < NI:
            xceT_next = prep(i + 1)

        # scores + softmax-numerator + AV accumulate
        for hp in range(NHP):
            nh = 1 if hp == 12 else 2
            st = psS.tile([128, 4, CH], f32, name="st", tag="st")
            for hh in range(nh):
                p0 = hh * 64
                for lc in range(2):
                    te.matmul(st[:, hh * 2 + lc, :],
                              lhsT=kt[p0:p0 + 64, hp, lc * 128:(lc + 1) * 128],
                              rhs=qst[p0:p0 + 64, hp, :],
                              start=True, stop=True)
            ptm = pool_sc.tile([128, 4, CH], f16, name="ptm", tag="ptm")
            act.activation(out=ptm[:, 0:2 * nh, :], in_=st[:, 0:2 * nh, :],
                           func=Act.Exp)
            if i in (0, NI - 1):  # diagonal super-chunks: triangle mask
                for hh in range(nh):
                    vec.tensor_tensor(out=ptm[:, hh * 2:hh * 2 + 2, :],
                                      in0=ptm[:, hh * 2:hh * 2 + 2, :],
                                      in1=tri_sb, op=AluOp.mult)
            for hh in range(nh):
                h = hp * 2 + hh
                av = av_ring[:, h % 2, :]
                for lc in range(2):
                    te.matmul(av, lhsT=vt[:, lc, h, :],
                              rhs=ptm[:, hh * 2 + lc, :],
                              start=(lc == 0), stop=(lc == 1))
                vec.scalar_tensor_tensor(
                    out=acc[:, h, :], in0=acc[:, h, :],
                    scalar=mtab_sb[:, i:i + 1], in1=av,
                    op0=AluOp.mult, op1=AluOp.add)

        # predicated capture of chunk A's accumulator at i == p
        creg = sync.alloc_register(f"cp{i}")
        sync.reg_load(creg, ctab_sb[0:1, i:i + 1])
        cv = sync.snap(creg, donate=True, min_val=0, max_val=1)
        sync.dma_start(out=t["acap"], in_=acc.rearrange("p h q -> p (h q)"),
                       cond=cv, cond_hint=False)

    pool_sc.release()
    pool_it.release()
    pool_wkv.release()
    psK.release()
    psA.release()
    psS.release()

    # ================= P3: normalize -> attn_T =================
    sync.dma_start(out=capA.rearrange("p h q -> p (h q)"), in_=t["acap"])
    pool_y = tc.alloc_tile_pool(name="pool_y", bufs=1, side="right")
    y = [pool_y.tile([128, D], f32, name=f"y{tt}", tag=f"y{tt}")
         for tt in range(4)]
    for tt in range(4):
        act.dma_start(out=y[tt], in_=t["xq"][tt * 128:(tt + 1) * 128, :])
    pool_fin = tc.alloc_tile_pool(name="pool_fin", bufs=1, side="right")
    attn_T = [pool_fin.tile([128, TOK], f16, name=f"aT{i}", tag=f"aT{i}")
              for i in range(NHP)]
    pool_nrm = tc.alloc_tile_pool(name="pool_nrm", bufs=2)
    psN = tc.alloc_tile_pool(name="psN", bufs=1, space="PSUM")
    den_ring = psN.tile([64, 2, 2 * CH], f32, name="den_ring")
    NG = [(h0, min(2, H - h0)) for h0 in range(0, H, 2)]
    gctr = 0
    for asrc, coff in ((acc, CH), (capA, 0)):  # B first: overlaps capA DMA
        for h0, gh in NG:
            rcp = pool_nrm.tile([1, 2, CH], f16, name="rcp", tag="rcp")
            with nc.allow_low_precision(
                    reason="fp16 softmax denominators, rel err ~5e-4"):
                vec.reciprocal(out=rcp[:, :gh, :],
                               in_=asrc[64:65, h0:h0 + gh, :])
            den = den_ring[:, gctr % 2, :gh * CH]
            te.matmul(den, lhsT=ones_h,
                      rhs=rcp.rearrange("p g c -> p (g c)")[:, :gh * CH],
                      start=True, stop=True)
            gctr += 1
            dview = den.rearrange("p (g c) -> p g c", c=CH)
            for k in range(gh):
                h = h0 + k
                vec.tensor_tensor(
                    out=attn_T[h // 2][(h % 2) * 64:(h % 2) * 64 + 64,
                                       coff:coff + CH],
                    in0=asrc[0:64, h, :], in1=dview[:, k, :], op=AluOp.mult)
    psN.release()
    pool_nrm.release()

    # ================= P4: proj + residual =================
    bpj_sb = pool_y.tile([128, D], f32, name="bpj_sb")
    act.dma_start(out=bpj_sb, in_=bass.AP(
        tensor=t["bpj"].tensor, offset=0, ap=[[0, 128], [1, D]]))
    bo_sb = pool_y.tile([128, D], f32, name="bo_sb")
    act.dma_start(out=bo_sb, in_=bass.AP(
        tensor=t["bo"].tensor, offset=0, ap=[[0, 128], [1, D]]))

    psC = tc.alloc_tile_pool(name="psC", bufs=4, space="PSUM")
    psD = tc.alloc_tile_pool(name="psD", bufs=3, space="PSUM")
    pool_pw = tc.alloc_tile_pool(name="pool_pw", bufs=3)
    pps = {}
    for j0, (c0, cw) in enumerate(NJ):
        for ci, (d0, dp) in enumerate(DCH):
            wpt = pool_pw.tile([128, 400], f16, name=f"wp{j0}_{ci}",
                               tag="w400", bufs=6)
            sync.dma_start(out=wpt, in_=t["wp"][j0, :, ci, :])
            for tt in range(4):
                if ci == 0:
                    pps[tt] = psC.tile([128, 400], f32, name=f"pps{tt}",
                                       tag="psc")
                te.matmul(pps[tt], lhsT=attn_T[ci][:dp, tt * 128:(tt + 1) * 128],
                          rhs=wpt[:dp, :], start=(ci == 0),
                          stop=(ci == len(DCH) - 1))
        for tt in range(4):
            vec.tensor_tensor(out=y[tt][:, c0:c0 + cw],
                              in0=y[tt][:, c0:c0 + cw], in1=pps[tt],
                              op=AluOp.add)
            vec.tensor_tensor(out=y[tt][:, c0:c0 + cw],
                              in0=y[tt][:, c0:c0 + cw],
                              in1=bpj_sb[:, c0:c0 + cw], op=AluOp.add)
    pool_pw.release()

    # ================= P5: LN2 + MLP =================
    pool_m = tc.alloc_tile_pool(name="pool_m", bufs=1)
    ycT = [pool_m.tile([128, TOK], f16, name=f"ycT{i}", tag=f"ycT{i}")
           for i in range(NHP)]
    pool_ln2 = tc.alloc_tile_pool(name="pool_ln2", bufs=2)
    for tt in range(4):
        yc = layernorm_cast(pool_ln2, y[tt], "ln2")
        transpose_into(yc, lambda ci: ycT[ci], tt * 128)
    pool_ln2.release()

    pool_h = tc.alloc_tile_pool(name="pool_h", bufs=2)
    pool_w2 = tc.alloc_tile_pool(name="pool_w2", bufs=3)
    ops = {}
    f_base = 0
    for ng in GRP:
        hT = [pool_h.tile([128, TOK], f16, name=f"hT{f_base}_{fi}",
                          tag=f"hT{fi}") for fi in range(ng)]
        for fi in range(ng):
            f = f_base + fi
            wft = pool_w2.tile([128, NHP, 128], f16, name=f"wf{f}", tag="wfc")
            sync.dma_start(out=wft, in_=t["wfc"][f])
            ps = psD.tile([128, TOK], f32, name="hps", tag="psd")
            for ci, (d0, dp) in enumerate(DCH):
                te.matmul(ps, lhsT=wft[:dp, ci, :], rhs=ycT[ci][:dp, :],
                          start=(ci == 0), stop=(ci == len(DCH) - 1))
            act.activation(out=hT[fi], in_=ps, func=Act.Gelu_apprx_tanh,
                           bias=bfc_sb[:, f:f + 1], scale=1.0)
        last = (f_base + ng == 50)
        for j0, (c0, cw) in enumerate(NJ):
            for fi in range(ng):
                f = f_base + fi
                wot = pool_w2.tile([128, 400], f16, name=f"wo{f}_{j0}",
                                   tag="wo400", bufs=6)
                sync.dma_start(out=wot, in_=t["wo"][j0, f, :, :])
                for tt in range(4):
                    if fi == 0:
                        ops[tt] = psC.tile([128, 400], f32, name=f"ops{tt}",
                                           tag="psc")
                    te.matmul(ops[tt], lhsT=hT[fi][:, tt * 128:(tt + 1) * 128],
                              rhs=wot, start=(fi == 0), stop=(fi == ng - 1))
            for tt in range(4):
                vec.tensor_tensor(out=y[tt][:, c0:c0 + cw],
                                  in0=y[tt][:, c0:c0 + cw], in1=ops[tt],
                                  op=AluOp.add)
                if last:  # final bias + store, column-wise to shrink the tail
                    vec.tensor_tensor(out=y[tt][:, c0:c0 + cw],
                                      in0=y[tt][:, c0:c0 + cw],
                                      in1=bo_sb[:, c0:c0 + cw], op=AluOp.add)
                    sync.dma_start(
                        out=t["out"][tt * 128:(tt + 1) * 128, c0:c0 + cw],
                        in_=y[tt][:, c0:c0 + cw])
        f_base += ng

    pool_w2.release()
    pool_h.release()
    pool_m.release()
    pool_fin.release()
    pool_y.release()
    pool_acc.release()
    persist.release()
    psD.release()
    psC.release()
    psT.release()


_cached_nc = None


def _get_nc():
    global _cached_nc
    if _cached_nc is None:
        _cached_nc = _build()
    return _cached_nc


def _pack_ct(w, tile_w):
    """(K, N) f32 -> (ceil(N/tile_w), 128, 13, tile_w) f16, contract
    zero-padded to 1664 rows and output zero-padded to a tile multiple,
    laid out for contiguous per-tile loads."""
    K, N = w.shape
    nt = -(-N // tile_w)
    wp = np.zeros((NHP * 128, nt * tile_w), np.float16)
    wp[:K, :N] = w.astype(np.float16)
    wp = wp.reshape(NHP, 128, nt * tile_w)
    out = np.empty((nt, 128, NHP, tile_w), np.float16)
    for tq in range(nt):
        out[tq] = wp[:, :, tq * tile_w:(tq + 1) * tile_w].transpose(1, 0, 2)
    return np.ascontiguousarray(out)


def _host_common(g1, b1, w_qkv, bias_qkv, w_proj, bias_proj, g2, b2, w_fc,
                 bias_fc, w_out, bias_out):
    w_qkv = np.asarray(w_qkv, np.float32)
    wm = w_qkv * np.asarray(g1, np.float32)[:, None]
    bm = (np.asarray(bias_qkv, np.float32)
          + np.asarray(b1, np.float32) @ w_qkv)
    sc = 1.0 / np.sqrt(C)
    wm[:, :D] *= sc
    bm[:D] *= sc
    wfc_m = np.asarray(w_fc, np.float32) * np.asarray(g2, np.float32)[:, None]
    bfc_m = (np.asarray(bias_fc, np.float32)
             + np.asarray(b2, np.float32) @ np.asarray(w_fc, np.float32))

    # v padded to 65-wide heads, pad col bias = 1 (softmax denominator)
    wv = wm[:, 2 * D:3 * D].reshape(D, H, C)
    wv_pad = np.zeros((D, H, 65), np.float32)
    wv_pad[:, :, :C] = wv
    bv_pad = np.zeros((H, 65), np.float32)
    bv_pad[:, :C] = bm[2 * D:3 * D].reshape(H, C)
    bv_pad[:, C] = 1.0

    def colpack(b, n):
        bp = np.zeros((n * 128,), np.float32)
        bp[:len(b)] = b
        return np.ascontiguousarray(bp.reshape(n, 128).T)

    tri = (np.arange(256)[:, None] <= np.arange(CH)[None, :]).astype(
        np.float16).reshape(2, 128, CH).transpose(1, 0, 2)

    wo = np.asarray(w_out, np.float32).reshape(50, 128, D)
    wo_t = np.stack([np.ascontiguousarray(wo[:, :, c0:c0 + cw])
                     for c0, cw in NJ])

    return {
        "wq": _pack_ct(wm[:, 0:D], 128),
        "wk": _pack_ct(wm[:, D:2 * D], 128),
        "wv": _pack_ct(wv_pad.reshape(D, V65), VW),
        "wp": _pack_ct(np.asarray(w_proj, np.float32), 400),
        "wfc": _pack_ct(wfc_m, 128),
        "wo": np.ascontiguousarray(wo_t.astype(np.float16)),
        "bq": colpack(bm[0:D], NHP),
        "bk": colpack(bm[D:2 * D], NHP),
        "bv": np.ascontiguousarray(
            bv_pad.reshape(1, V65).astype(np.float16)),
        "bpj": np.asarray(bias_proj, np.float32).reshape(1, D),
        "bfc": colpack(bfc_m, 50),
        "bo": np.asarray(bias_out, np.float32).reshape(1, D),
        "tri": np.ascontiguousarray(tri),
    }


def kernel(x, g1, b1, w_qkv, bias_qkv, w_proj, bias_proj, g2, b2, w_fc,
           bias_fc, w_out, bias_out):
    x = np.asarray(x, np.float32)
    xf = x.reshape(B * S, D)
    common = _host_common(g1, b1, w_qkv, bias_qkv, w_proj, bias_proj, g2, b2,
                          w_fc, bias_fc, w_out, bias_out)

    in_maps = []
    for j in range(N_CORES):
        p = j
        a0 = CH * p
        b0 = S + CH * (7 - p)
        xq = np.concatenate([xf[a0:a0 + CH], xf[b0:b0 + CH]], axis=0)
        # iteration order: A-diag, A full prefix, B full prefix, B-diag
        supers = [a0] + [CH * s for s in range(p)] \
            + [S + CH * s for s in range(7 - p)] + [b0]
        xext = np.concatenate([xf[r0:r0 + CH] for r0 in supers], axis=0)
        qoff = np.zeros((1, NI), np.int32)
        mtab = np.ones((65, NI), np.float32)
        ctab = np.zeros((1, NI), np.int32)
        for i in range(NI):
            qoff[0, i] = 0 if i <= p else 128 * QROW
        mtab[:, 0] = 0.0
        if p + 1 < NI:
            mtab[:, p + 1] = 0.0
        ctab[0, p] = 1
        in_maps.append({
            "xq": np.ascontiguousarray(xq),
            "xext": np.ascontiguousarray(xext.astype(np.float16)),
            "qoff": qoff, "mtab": mtab, "ctab": ctab,
            **common,
        })

    nc = _get_nc()
    res = run_bass_kernel_spmd(nc, in_maps, core_ids=list(range(N_CORES)))

    of = np.empty((B * S, D), np.float32)
    for j in range(N_CORES):
        o = res.results[j]["out"]
        a0 = CH * j
        b0 = S + CH * (7 - j)
        of[a0:a0 + CH] = o[:CH]
        of[b0:b0 + CH] = o[CH:]
    return of.reshape(B, S, D)
